# revision 5
# baseline (speedup 1.0000x reference)
"""MoE MLP (GPT-2 style experts, top-2 routing) on 8 Trainium2 NeuronCores.

Strategy (expert-parallel, per sharding hint):
  - Host: router matmul + softmax + top-2 + renormalize (tiny: N x 1024 @ 1024 x 8).
  - Host: dispatch tokens by expert id -> per-core gathered token block (all-to-all
    realized at the shard step), transposed to [C, M] so the device kernel only
    does natural-layout matmuls.
  - Device (core e): outT = w_proj[e].T @ gelu(w_fc[e].T @ xT + b_fc[e])
    computed as tiled PE matmuls, fp16 inputs with f32 PSUM accumulation.
    Both weight matrices stay resident in SBUF (fp16).
  - Host: combine: out[tok] += gate * (y + b_proj[e]) for each routed pair.
"""

import functools
import os

import numpy as np

import concourse.bacc as bacc
import concourse.mybir as mybir
import concourse.tile as tile
from concourse.bass_utils import run_bass_kernel_spmd

N_EMBD = 1024
D_FF = 4096
N_EXPERTS = 8
TOP_K = 2
N_CORES = 8
P = 128
KT = N_EMBD // P      # 8 k-tiles (contraction over n_embd)
FT = D_FF // P        # 32 ff-tiles (contraction over d_ff for proj)
CT = N_EMBD // P      # 8 output-channel tiles
MSZ = 512             # moving (token) tile width

DT16 = mybir.dt.float16
F32 = mybir.dt.float32


def _m_tiles(M, msz=MSZ, tail_min=256):
    out = []
    m0 = 0
    while m0 < M:
        out.append((m0, min(msz, M - m0)))
        m0 += msz
    # A tail tile narrower than ~256 columns can't hide the per-matmul
    # LDWEIGHTS (~53ns) under its stream time; rebalance the last two
    # tiles so both are >= tail_min wide.
    if len(out) >= 2 and 0 < out[-1][1] < tail_min:
        (m0a, wa), (m0b, wb) = out[-2], out[-1]
        steal = tail_min - wb
        out[-2] = (m0a, wa - steal)
        out[-1] = (m0b - steal, wb + steal)
    return out


@functools.lru_cache(maxsize=16)
def _build(M, repeat=1, act_identity=False, msz=MSZ, psa_bufs=3, psb_bufs=3,
           x_bufs=2, h_bufs=1, o_bufs=4, weights_in_loop=False, tail_min=256):
    """Bass program: per-core dense expert MLP over M gathered tokens."""
    act_fn = (mybir.ActivationFunctionType.Identity if act_identity
              else mybir.ActivationFunctionType.Gelu)
    nc = bacc.Bacc("TRN2", target_bir_lowering=False, debug=False)

    xT = nc.dram_tensor("xT", [KT, P, M], DT16, kind="ExternalInput")
    wfc = nc.dram_tensor("w_fc", [KT, P, D_FF], DT16, kind="ExternalInput")
    bfcT = nc.dram_tensor("b_fcT", [P, FT], F32, kind="ExternalInput")
    wproj = nc.dram_tensor("w_proj", [FT, P, N_EMBD], DT16, kind="ExternalInput")
    outT = nc.dram_tensor("outT", [CT, P, M], F32, kind="ExternalOutput")

    with tile.TileContext(nc) as tc:
        with tc.tile_pool(name="weights", bufs=1) as wpool, \
             tc.tile_pool(name="xp", bufs=x_bufs) as xpool, \
             tc.tile_pool(name="hp", bufs=h_bufs) as hpool, \
             tc.tile_pool(name="op", bufs=o_bufs) as opool, \
             tc.tile_pool(name="psA", bufs=psa_bufs, space="PSUM") as psA, \
             tc.tile_pool(name="psB", bufs=psb_bufs, space="PSUM") as psB:

            def load_x(m0, mw):
                x_sb = xpool.tile([P, KT, msz], DT16, tag="x", name="x_sb")
                for k in range(KT):
                    nc.sync.dma_start(x_sb[:, k, :mw], xT[k, :, m0:m0 + mw])
                return x_sb

            tiles = _m_tiles(M, msz, tail_min)
            # First token tile queued ahead of the weights so the PE can
            # start as soon as the first w_fc column-chunk lands.
            pre_x = load_x(*tiles[0])

            def load_weights():
                wfc_sb = wpool.tile([P, KT, D_FF], DT16, tag="wfc",
                                    name="wfc_sb")
                CHUNK = 1024
                for c0 in range(0, D_FF, CHUNK):
                    for k in range(KT):
                        nc.sync.dma_start(
                            wfc_sb[:, k, c0:c0 + CHUNK],
                            wfc[k, :, c0:c0 + CHUNK]
                        )
                bfc_sb = wpool.tile([P, FT], F32, tag="bfc", name="bfc_sb")
                nc.sync.dma_start(bfc_sb[:, :], bfcT[:, :])
                wproj_sb = wpool.tile([P, FT, N_EMBD], DT16, tag="wproj",
                                      name="wproj_sb")
                for f in range(FT):
                    nc.sync.dma_start(wproj_sb[:, f, :], wproj[f, :, :])
                return wfc_sb, bfc_sb, wproj_sb

            if not weights_in_loop:
                wfc_sb, bfc_sb, wproj_sb = load_weights()

            for _r in range(repeat):
                if weights_in_loop:
                    wfc_sb, bfc_sb, wproj_sb = load_weights()
                for ti, (m0, mw) in enumerate(tiles):
                    x_sb = pre_x if (_r == 0 and ti == 0) else load_x(m0, mw)

                    hT_sb = hpool.tile([P, FT, msz], DT16, tag="h")
                    for f in range(FT):
                        ps = psA.tile([P, msz], F32, tag="psA")
                        for k in range(KT):
                            nc.tensor.matmul(
                                ps[:, :mw],
                                wfc_sb[:, k, f * P:(f + 1) * P],
                                x_sb[:, k, :mw],
                                start=(k == 0),
                                stop=(k == KT - 1),
                            )
                        nc.scalar.activation(
                            hT_sb[:, f, :mw], ps[:, :mw],
                            act_fn,
                            bias=bfc_sb[:, f:f + 1],
                        )

                    for c in range(CT):
                        ps2 = psB.tile([P, msz], F32, tag="psB")
                        for f in range(FT):
                            nc.tensor.matmul(
                                ps2[:, :mw],
                                wproj_sb[:, f, c * P:(c + 1) * P],
                                hT_sb[:, f, :mw],
                                start=(f == 0),
                                stop=(f == FT - 1),
                            )
                        o_sb = opool.tile([P, msz], F32, tag="o")
                        nc.vector.tensor_copy(o_sb[:, :mw], ps2[:, :mw])
                        nc.sync.dma_start(outT[c, :, m0:m0 + mw], o_sb[:, :mw])

    nc.compile()
    return nc


def _route(x_flat, router_w):
    """Top-2 routing, matching the reference numerics (f32)."""
    N = x_flat.shape[0]
    logits = x_flat @ router_w.T                      # [N, E]
    logits -= logits.max(axis=-1, keepdims=True)
    p = np.exp(logits)
    p /= p.sum(axis=-1, keepdims=True)
    rows = np.arange(N)
    i1 = p.argmax(axis=-1)
    p1 = p[rows, i1]
    pm = p.copy()
    pm[rows, i1] = -1.0
    i2 = pm.argmax(axis=-1)
    p2 = p[rows, i2]
    s = p1 + p2 + 1e-9
    return i1, i2, p1 / s, p2 / s


def kernel(x, router_w, w_fc, b_fc, w_proj, b_proj):
    x = np.asarray(x, dtype=np.float32)
    router_w = np.asarray(router_w, dtype=np.float32)
    w_fc = np.asarray(w_fc, dtype=np.float32)
    b_fc = np.asarray(b_fc, dtype=np.float32)
    w_proj = np.asarray(w_proj, dtype=np.float32)
    b_proj = np.asarray(b_proj, dtype=np.float32)

    B, T, C = x.shape
    x_flat = x.reshape(-1, C)
    N = x_flat.shape[0]

    i1, i2, g1, g2 = _route(x_flat, router_w)

    idxs, gates = [], []
    for e in range(N_EXPERTS):
        mask = (i1 == e) | (i2 == e)
        idx = np.flatnonzero(mask)
        g = np.where(i1[idx] == e, g1[idx], g2[idx]).astype(np.float32)
        idxs.append(idx)
        gates.append(g)

    max_cnt = max(len(ix) for ix in idxs)
    M = max(P, ((max_cnt + P - 1) // P) * P)

    repeat = int(os.environ.get("MOE_KERNEL_REPEAT", "1"))
    nc = _build(M, repeat)

    in_maps = []
    for e in range(N_EXPERTS):
        idx = idxs[e]
        xg = np.zeros((M, C), dtype=np.float32)
        xg[: len(idx)] = x_flat[idx]
        xT = np.ascontiguousarray(xg.T).reshape(KT, P, M).astype(np.float16)
        in_maps.append({
            "xT": xT,
            "w_fc": w_fc[e].reshape(KT, P, D_FF).astype(np.float16),
            "b_fcT": np.ascontiguousarray(b_fc[e].reshape(FT, P).T),
            "w_proj": w_proj[e].reshape(FT, P, N_EMBD).astype(np.float16),
        })

    res = run_bass_kernel_spmd(nc, in_maps, core_ids=list(range(N_CORES)))

    out_flat = np.zeros((N, C), dtype=np.float32)
    for e in range(N_EXPERTS):
        idx = idxs[e]
        yT = res.results[e]["outT"].reshape(C, M)       # [C, M]
        y = yT.T[: len(idx)]                            # [n_e, C]
        out_flat[idx] += gates[e][:, None] * (y + b_proj[e])

    return out_flat.reshape(B, T, C)



# revision 9
# speedup vs baseline: 1.1686x; 1.1686x over previous
"""MoE MLP (GPT-2 style experts, top-2 routing) on 8 Trainium2 NeuronCores.

Strategy (expert-parallel, per sharding hint):
  - Host: router matmul + softmax + top-2 + renormalize (tiny: N x 1024 @ 1024 x 8).
  - Host: dispatch tokens by expert id -> per-core gathered token block (all-to-all
    realized at the shard step), transposed to [C, M] so the device kernel only
    does natural-layout matmuls.
  - Device (core e): outT = w_proj[e].T @ gelu(w_fc[e].T @ xT + b_fc[e])
    as tiled PE matmuls, fp16 operands with f32 PSUM accumulation.
  - Host: combine: out[tok] += gate * (y + b_proj[e]) for each routed pair.

Performance structure (measured on trn2 via axon):
  - The PE roofline for this shape is ~464us/core/pass, but each matmul paid
    ~70ns of serialized LDWEIGHTS + semaphore overhead (measured with a
    back-to-back matmul probe: 282ns vs the 216ns stream model at N=512),
    putting the naive schedule at ~680us.
  - Fix 1: process a GROUP of m-tiles per stationary-weight load: the k/f
    weight tile is loaded once and streamed against each tile's tokens, and
    `_optimize_pe_stream` drops the now-redundant adjacent InstLdweights
    (walrus's own redundant-LDW pass rejects explicit InstLdweights, so the
    dedup happens here, pre-compile). LDW count: one per matmul -> one per
    group (2560 -> ~1024 per pass).
  - (A sem-inc batching pass exists but is disabled: walrus asserts
    UpdateValue == 1 on semaphore updates, so per-matmul increments stay.)
  - To fit group=3 hT buffers in SBUF, w_fc streams from DRAM as per-f slabs
    (2KB/partition each, triple buffered) while w_proj stays resident.
  - A tail tile narrower than 256 columns cannot hide even a deduped LDW
    under its stream time, so the last two tiles are rebalanced to >=256.
"""

import functools

import numpy as np

import concourse.bacc as bacc
import concourse.mybir as mybir
import concourse.tile as tile
from concourse.bass_utils import run_bass_kernel_spmd

N_EMBD = 1024
D_FF = 4096
N_EXPERTS = 8
TOP_K = 2
N_CORES = 8
P = 128
KT = N_EMBD // P      # 8 k-tiles (contraction over n_embd)
FT = D_FF // P        # 32 ff-tiles (contraction over d_ff for proj)
CT = N_EMBD // P      # 8 output-channel tiles
MSZ = 512             # moving (token) tile width

DT16 = mybir.dt.float16
F32 = mybir.dt.float32


def _m_tiles(M, msz=MSZ, tail_min=256):
    out = []
    m0 = 0
    while m0 < M:
        out.append((m0, min(msz, M - m0)))
        m0 += msz
    if len(out) >= 2 and 0 < out[-1][1] < tail_min:
        (m0a, wa), (m0b, wb) = out[-2], out[-1]
        steal = tail_min - wb
        out[-2] = (m0a, wa - steal)
        out[-1] = (m0b - steal, wb + steal)
    return out


def _optimize_pe_stream(nc, dedup=True, incbatch=False, inc_cap=200):
    """Post-schedule, pre-compile cleanup of the PE instruction stream.

    dedup: drop an InstLdweights whose stationary operand (memref, offset,
    access pattern, dtype, mode) matches the previous PE weight load, when
    nothing in between could have modified that SBUF region (only PE
    matmuls, or instructions writing other memrefs). Sync info of a dropped
    load is merged into the next matmul (waits moved earlier/kept: safe).

    incbatch: PE matmuls each carry a single sem-inc(1) used by consumers
    (ACT/DVE) to track progress. Runs of such matmuls are rewritten so only
    the last matmul of the run increments, by the run's total. The run is
    flushed before any PE instruction that waits (so every PE wait observes
    the same semaphore state as before), and at a cap / block end. Consumers
    observe increments slightly later -> strictly conservative.
    """
    PE = mybir.EngineType.PE
    for fn in nc.m.functions:
        for blk in fn.blocks:
            ins = list(blk.instructions)
            changed = False

            if dedup:
                out = []
                last_sig = None
                pend_wait, pend_upd = [], []
                for i in ins:
                    tn = type(i).__name__
                    eng = getattr(i, "engine", None)
                    if tn == "InstLdweights" and eng == PE:
                        ap = i.ins[0]
                        sig = (
                            ap.memref, ap.offset, str(ap.ap), str(ap.dtype),
                            str(getattr(i, "perf_mode", None)),
                            str(getattr(i, "is_transpose", None)),
                            str(getattr(i, "tile_position", None)),
                        )
                        if last_sig is not None and sig == last_sig:
                            si = i.sync_info
                            if si is not None:
                                pend_wait += list(si.on_wait)
                                pend_upd += list(si.on_update)
                            changed = True
                            continue
                        last_sig = sig
                        out.append(i)
                    elif tn == "InstMatmult" and eng == PE:
                        if pend_wait or pend_upd:
                            si = i.sync_info
                            ow = list(si.on_wait) if si else []
                            ou = list(si.on_update) if si else []
                            i.sync_info = mybir.SyncInfo(
                                on_wait=pend_wait + ow, on_update=pend_upd + ou
                            )
                            pend_wait, pend_upd = [], []
                        out.append(i)
                    else:
                        if last_sig is not None:
                            if eng == PE:
                                last_sig = None
                            else:
                                for o in (getattr(i, "outs", None) or []):
                                    if getattr(o, "memref", None) == last_sig[0]:
                                        last_sig = None
                                        break
                        out.append(i)
                assert not pend_wait and not pend_upd, (
                    "dropped InstLdweights not followed by a matmul"
                )
                ins = out

            if incbatch:
                pend_mm = None
                pend_cnt = 0
                pend_sem = None  # (sync_type, id, ant_name)

                def flush():
                    nonlocal pend_mm, pend_cnt
                    if pend_mm is not None and pend_cnt > 0:
                        si = pend_mm.sync_info
                        upd = mybir.SyncUpdate(
                            sync_type=pend_sem[0], id=pend_sem[1],
                            ant_name=pend_sem[2], update_mode="sem-inc",
                            update_value=pend_cnt, update_reg=None,
                        )
                        pend_mm.sync_info = mybir.SyncInfo(
                            on_wait=list(si.on_wait) if si else [],
                            on_update=[upd],
                        )
                    pend_mm = None
                    pend_cnt = 0

                for i in ins:
                    eng = getattr(i, "engine", None)
                    if eng != PE:
                        continue
                    tn = type(i).__name__
                    si = i.sync_info
                    if si is not None and len(si.on_wait) > 0:
                        flush()
                    if tn != "InstMatmult":
                        continue
                    upds = list(si.on_update) if si else []
                    if (
                        len(upds) == 1
                        and upds[0].update_mode == "sem-inc"
                        and upds[0].update_value == 1
                        and upds[0].update_reg is None
                    ):
                        sem = (upds[0].sync_type, upds[0].id, upds[0].ant_name)
                        if pend_sem is not None and sem != pend_sem:
                            flush()
                        pend_sem = sem
                        i.sync_info = mybir.SyncInfo(
                            on_wait=list(si.on_wait) if si else [], on_update=[]
                        )
                        pend_mm = i
                        pend_cnt += 1
                        changed = True
                        if pend_cnt >= inc_cap:
                            flush()
                    else:
                        flush()
                flush()

            if changed:
                blk.instructions = ins


@functools.lru_cache(maxsize=16)
def _build(M, repeat=1, group=3, msz=MSZ, psa_bufs=4, psb_bufs=4,
           x_bufs=3, o_bufs=4, wf_bufs=3, tail_min=256,
           dedup=True, incbatch=False):
    """Bass program: per-core dense expert MLP over M gathered tokens.

    Processes `group` m-tiles per stationary weight load so the LDW dedup
    pass can elide all but the first load of each group.
    """
    nc = bacc.Bacc("TRN2", target_bir_lowering=False, debug=False)

    xT = nc.dram_tensor("xT", [KT, P, M], DT16, kind="ExternalInput")
    wfcT = nc.dram_tensor("wfcT", [FT, P, KT * P], DT16, kind="ExternalInput")
    bfcT = nc.dram_tensor("b_fcT", [P, FT], F32, kind="ExternalInput")
    wproj = nc.dram_tensor("w_proj", [FT, P, N_EMBD], DT16, kind="ExternalInput")
    outT = nc.dram_tensor("outT", [CT, P, M], F32, kind="ExternalOutput")

    tiles = _m_tiles(M, msz, tail_min)
    groups = [tiles[i:i + group] for i in range(0, len(tiles), group)]

    with tile.TileContext(nc) as tc:
        with tc.tile_pool(name="weights", bufs=1) as wpool, \
             tc.tile_pool(name="wfp", bufs=wf_bufs) as wfpool, \
             tc.tile_pool(name="xp", bufs=x_bufs) as xpool, \
             tc.tile_pool(name="hp", bufs=1) as hpool, \
             tc.tile_pool(name="op", bufs=o_bufs) as opool, \
             tc.tile_pool(name="psA", bufs=psa_bufs, space="PSUM") as psA, \
             tc.tile_pool(name="psB", bufs=psb_bufs, space="PSUM") as psB:

            def load_x(m0, mw):
                x_sb = xpool.tile([P, KT, msz], DT16, tag="x", name="x_sb")
                for k in range(KT):
                    nc.sync.dma_start(x_sb[:, k, :mw], xT[k, :, m0:m0 + mw])
                return x_sb

            # First group's tokens queued ahead of the resident weights so
            # the PE can start as soon as the first wfc slab lands.
            pre_x = [load_x(m0, mw) for (m0, mw) in groups[0]]

            bfc_sb = wpool.tile([P, FT], F32, tag="bfc", name="bfc_sb")
            nc.sync.dma_start(bfc_sb[:, :], bfcT[:, :])
            wproj_sb = wpool.tile([P, FT, N_EMBD], DT16, tag="wproj",
                                  name="wproj_sb")
            for f in range(FT):
                nc.sync.dma_start(wproj_sb[:, f, :], wproj[f, :, :])

            for _r in range(repeat):
                for gi, g in enumerate(groups):
                    if _r == 0 and gi == 0:
                        xs = pre_x
                    else:
                        xs = [load_x(m0, mw) for (m0, mw) in g]
                    hs = [hpool.tile([P, FT, msz], DT16, tag=f"h{ti}", name=f"h{ti}_sb")
                          for ti in range(len(g))]

                    for f in range(FT):
                        wf = wfpool.tile([P, KT * P], DT16, tag="wf", name="wf_sb")
                        nc.sync.dma_start(wf[:, :], wfcT[f, :, :])
                        pss = [psA.tile([P, msz], F32, tag="psA", name="psA_t")
                               for _ in g]
                        for k in range(KT):
                            for ti, (m0, mw) in enumerate(g):
                                nc.tensor.matmul(
                                    pss[ti][:, :mw],
                                    wf[:, k * P:(k + 1) * P],
                                    xs[ti][:, k, :mw],
                                    start=(k == 0),
                                    stop=(k == KT - 1),
                                )
                        for ti, (m0, mw) in enumerate(g):
                            nc.scalar.activation(
                                hs[ti][:, f, :mw], pss[ti][:, :mw],
                                mybir.ActivationFunctionType.Gelu,
                                bias=bfc_sb[:, f:f + 1],
                            )

                    for c in range(CT):
                        pss = [psB.tile([P, msz], F32, tag="psB", name="psB_t")
                               for _ in g]
                        for f in range(FT):
                            for ti, (m0, mw) in enumerate(g):
                                nc.tensor.matmul(
                                    pss[ti][:, :mw],
                                    wproj_sb[:, f, c * P:(c + 1) * P],
                                    hs[ti][:, f, :mw],
                                    start=(f == 0),
                                    stop=(f == FT - 1),
                                )
                        for ti, (m0, mw) in enumerate(g):
                            o_sb = opool.tile([P, msz], F32, tag="o", name="o_sb")
                            nc.vector.tensor_copy(o_sb[:, :mw], pss[ti][:, :mw])
                            nc.sync.dma_start(outT[c, :, m0:m0 + mw],
                                              o_sb[:, :mw])

    _optimize_pe_stream(nc, dedup=dedup, incbatch=incbatch)
    nc.compile()
    return nc


def _route(x_flat, router_w):
    """Top-2 routing, matching the reference numerics (f32)."""
    N = x_flat.shape[0]
    logits = x_flat @ router_w.T                      # [N, E]
    logits -= logits.max(axis=-1, keepdims=True)
    p = np.exp(logits)
    p /= p.sum(axis=-1, keepdims=True)
    rows = np.arange(N)
    i1 = p.argmax(axis=-1)
    p1 = p[rows, i1]
    pm = p.copy()
    pm[rows, i1] = -1.0
    i2 = pm.argmax(axis=-1)
    p2 = p[rows, i2]
    s = p1 + p2 + 1e-9
    return i1, i2, p1 / s, p2 / s


def _prep_in_maps(x_flat, idxs, w_fc, b_fc, w_proj, M):
    """Per-core input dict for core e = expert e (gathered, padded, f16)."""
    C = x_flat.shape[1]
    in_maps = []
    for e in range(N_EXPERTS):
        idx = idxs[e]
        xg = np.zeros((M, C), dtype=np.float32)
        xg[: len(idx)] = x_flat[idx]
        wfcT = np.ascontiguousarray(
            w_fc[e].reshape(KT, P, FT, P).transpose(2, 1, 0, 3)
        ).reshape(FT, P, KT * P).astype(np.float16)
        in_maps.append({
            "xT": np.ascontiguousarray(xg.T).reshape(KT, P, M).astype(np.float16),
            "wfcT": wfcT,
            "b_fcT": np.ascontiguousarray(b_fc[e].reshape(FT, P).T),
            "w_proj": w_proj[e].reshape(FT, P, N_EMBD).astype(np.float16),
        })
    return in_maps


def kernel(x, router_w, w_fc, b_fc, w_proj, b_proj):
    x = np.asarray(x, dtype=np.float32)
    router_w = np.asarray(router_w, dtype=np.float32)
    w_fc = np.asarray(w_fc, dtype=np.float32)
    b_fc = np.asarray(b_fc, dtype=np.float32)
    w_proj = np.asarray(w_proj, dtype=np.float32)
    b_proj = np.asarray(b_proj, dtype=np.float32)

    B, T, C = x.shape
    x_flat = x.reshape(-1, C)
    N = x_flat.shape[0]

    i1, i2, g1, g2 = _route(x_flat, router_w)

    idxs, gates = [], []
    for e in range(N_EXPERTS):
        mask = (i1 == e) | (i2 == e)
        idx = np.flatnonzero(mask)
        g = np.where(i1[idx] == e, g1[idx], g2[idx]).astype(np.float32)
        idxs.append(idx)
        gates.append(g)

    max_cnt = max(len(ix) for ix in idxs)
    M = max(P, ((max_cnt + P - 1) // P) * P)

    nc = _build(M)
    in_maps = _prep_in_maps(x_flat, idxs, w_fc, b_fc, w_proj, M)
    res = run_bass_kernel_spmd(nc, in_maps, core_ids=list(range(N_CORES)))

    out_flat = np.zeros((N, C), dtype=np.float32)
    for e in range(N_EXPERTS):
        idx = idxs[e]
        yT = res.results[e]["outT"].reshape(C, M)       # [C, M]
        y = yT.T[: len(idx)]                            # [n_e, C]
        out_flat[idx] += gates[e][:, None] * (y + b_proj[e])

    return out_flat.reshape(B, T, C)


# revision 11
# speedup vs baseline: 1.1890x; 1.0174x over previous
"""MoE MLP (GPT-2 style experts, top-2 routing) on 8 Trainium2 NeuronCores.

Strategy (expert-parallel, per sharding hint):
  - Host: router matmul + softmax + top-2 + renormalize (tiny: N x 1024 @ 1024 x 8).
  - Host: dispatch tokens by expert id -> per-core gathered token block (all-to-all
    realized at the shard step), transposed to [C, M] so the device kernel only
    does natural-layout matmuls.
  - Device (core e): outT = w_proj[e].T @ gelu(w_fc[e].T @ xT + b_fc[e])
    as tiled PE matmuls, fp16 operands with f32 PSUM accumulation.
  - Host: combine: out[tok] += gate * (y + b_proj[e]) for each routed pair.

Performance structure (measured on trn2 via axon):
  - The PE roofline for this shape is ~464us/core/pass, but each matmul paid
    ~70ns of serialized LDWEIGHTS + semaphore overhead (measured with a
    back-to-back matmul probe: 282ns vs the 216ns stream model at N=512),
    putting the naive schedule at ~680us.
  - Fix 1: process a GROUP of m-tiles per stationary-weight load: the k/f
    weight tile is loaded once and streamed against each tile's tokens, and
    `_optimize_pe_stream` drops the now-redundant adjacent InstLdweights
    (walrus's own redundant-LDW pass rejects explicit InstLdweights, so the
    dedup happens here, pre-compile). LDW count: one per matmul -> one per
    group (2560 -> ~1024 per pass).
  - (A sem-inc batching pass exists but is disabled: walrus asserts
    UpdateValue == 1 on semaphore updates, so per-matmul increments stay.)
  - To fit group=3 hT buffers in SBUF, w_fc streams from DRAM as per-f slabs
    (2KB/partition each, triple buffered) while w_proj stays resident.
  - A tail tile narrower than 256 columns cannot hide even a deduped LDW
    under its stream time, so the last two tiles are rebalanced to >=256.
"""

import functools

import numpy as np

import concourse.bacc as bacc
import concourse.mybir as mybir
import concourse.tile as tile
from concourse.bass_utils import run_bass_kernel_spmd

N_EMBD = 1024
D_FF = 4096
N_EXPERTS = 8
TOP_K = 2
N_CORES = 8
P = 128
KT = N_EMBD // P      # 8 k-tiles (contraction over n_embd)
FT = D_FF // P        # 32 ff-tiles (contraction over d_ff for proj)
CT = N_EMBD // P      # 8 output-channel tiles
MSZ = 512             # moving (token) tile width

DT16 = mybir.dt.float16
F32 = mybir.dt.float32


def _m_tiles(M, msz=MSZ, tail_min=256):
    out = []
    m0 = 0
    while m0 < M:
        out.append((m0, min(msz, M - m0)))
        m0 += msz
    if len(out) >= 2 and 0 < out[-1][1] < tail_min:
        (m0a, wa), (m0b, wb) = out[-2], out[-1]
        steal = tail_min - wb
        out[-2] = (m0a, wa - steal)
        out[-1] = (m0b - steal, wb + steal)
    return out


def _optimize_pe_stream(nc, dedup=True, incbatch=False, inc_cap=200):
    """Post-schedule, pre-compile cleanup of the PE instruction stream.

    dedup: drop an InstLdweights whose stationary operand (memref, offset,
    access pattern, dtype, mode) matches the previous PE weight load, when
    nothing in between could have modified that SBUF region (only PE
    matmuls, or instructions writing other memrefs). Sync info of a dropped
    load is merged into the next matmul (waits moved earlier/kept: safe).

    incbatch: PE matmuls each carry a single sem-inc(1) used by consumers
    (ACT/DVE) to track progress. Runs of such matmuls are rewritten so only
    the last matmul of the run increments, by the run's total. The run is
    flushed before any PE instruction that waits (so every PE wait observes
    the same semaphore state as before), and at a cap / block end. Consumers
    observe increments slightly later -> strictly conservative.
    """
    PE = mybir.EngineType.PE
    for fn in nc.m.functions:
        for blk in fn.blocks:
            ins = list(blk.instructions)
            changed = False

            if dedup:
                out = []
                last_sig = None
                pend_wait, pend_upd = [], []
                for i in ins:
                    tn = type(i).__name__
                    eng = getattr(i, "engine", None)
                    if tn == "InstLdweights" and eng == PE:
                        ap = i.ins[0]
                        sig = (
                            ap.memref, ap.offset, str(ap.ap), str(ap.dtype),
                            str(getattr(i, "perf_mode", None)),
                            str(getattr(i, "is_transpose", None)),
                            str(getattr(i, "tile_position", None)),
                        )
                        if last_sig is not None and sig == last_sig:
                            si = i.sync_info
                            if si is not None:
                                pend_wait += list(si.on_wait)
                                pend_upd += list(si.on_update)
                            changed = True
                            continue
                        last_sig = sig
                        out.append(i)
                    elif tn == "InstMatmult" and eng == PE:
                        if pend_wait or pend_upd:
                            si = i.sync_info
                            ow = list(si.on_wait) if si else []
                            ou = list(si.on_update) if si else []
                            i.sync_info = mybir.SyncInfo(
                                on_wait=pend_wait + ow, on_update=pend_upd + ou
                            )
                            pend_wait, pend_upd = [], []
                        out.append(i)
                    else:
                        if last_sig is not None:
                            if eng == PE:
                                last_sig = None
                            else:
                                for o in (getattr(i, "outs", None) or []):
                                    if getattr(o, "memref", None) == last_sig[0]:
                                        last_sig = None
                                        break
                        out.append(i)
                assert not pend_wait and not pend_upd, (
                    "dropped InstLdweights not followed by a matmul"
                )
                ins = out

            if incbatch:
                pend_mm = None
                pend_cnt = 0
                pend_sem = None  # (sync_type, id, ant_name)

                def flush():
                    nonlocal pend_mm, pend_cnt
                    if pend_mm is not None and pend_cnt > 0:
                        si = pend_mm.sync_info
                        upd = mybir.SyncUpdate(
                            sync_type=pend_sem[0], id=pend_sem[1],
                            ant_name=pend_sem[2], update_mode="sem-inc",
                            update_value=pend_cnt, update_reg=None,
                        )
                        pend_mm.sync_info = mybir.SyncInfo(
                            on_wait=list(si.on_wait) if si else [],
                            on_update=[upd],
                        )
                    pend_mm = None
                    pend_cnt = 0

                for i in ins:
                    eng = getattr(i, "engine", None)
                    if eng != PE:
                        continue
                    tn = type(i).__name__
                    si = i.sync_info
                    if si is not None and len(si.on_wait) > 0:
                        flush()
                    if tn != "InstMatmult":
                        continue
                    upds = list(si.on_update) if si else []
                    if (
                        len(upds) == 1
                        and upds[0].update_mode == "sem-inc"
                        and upds[0].update_value == 1
                        and upds[0].update_reg is None
                    ):
                        sem = (upds[0].sync_type, upds[0].id, upds[0].ant_name)
                        if pend_sem is not None and sem != pend_sem:
                            flush()
                        pend_sem = sem
                        i.sync_info = mybir.SyncInfo(
                            on_wait=list(si.on_wait) if si else [], on_update=[]
                        )
                        pend_mm = i
                        pend_cnt += 1
                        changed = True
                        if pend_cnt >= inc_cap:
                            flush()
                    else:
                        flush()
                flush()

            if changed:
                blk.instructions = ins


@functools.lru_cache(maxsize=16)
def _build(M, repeat=1, group=3, msz=MSZ, psa_bufs=4, psb_bufs=4,
           x_bufs=3, o_bufs=4, wf_bufs=3, tail_min=256,
           dedup=True, incbatch=False):
    """Bass program: per-core dense expert MLP over M gathered tokens.

    Processes `group` m-tiles per stationary weight load so the LDW dedup
    pass can elide all but the first load of each group.
    """
    nc = bacc.Bacc("TRN2", target_bir_lowering=False, debug=False)

    xT = nc.dram_tensor("xT", [KT, P, M], DT16, kind="ExternalInput")
    wfcT = nc.dram_tensor("wfcT", [FT, P, KT * P], DT16, kind="ExternalInput")
    bfcT = nc.dram_tensor("b_fcT", [P, FT], F32, kind="ExternalInput")
    wproj = nc.dram_tensor("w_proj", [FT, P, N_EMBD], DT16, kind="ExternalInput")
    outT = nc.dram_tensor("outT", [CT, P, M], F32, kind="ExternalOutput")

    tiles = _m_tiles(M, msz, tail_min)
    groups = [tiles[i:i + group] for i in range(0, len(tiles), group)]

    with tile.TileContext(nc) as tc:
        with tc.tile_pool(name="weights", bufs=1) as wpool, \
             tc.tile_pool(name="wfp", bufs=wf_bufs) as wfpool, \
             tc.tile_pool(name="xp", bufs=x_bufs) as xpool, \
             tc.tile_pool(name="hp", bufs=1) as hpool, \
             tc.tile_pool(name="op", bufs=o_bufs) as opool, \
             tc.tile_pool(name="psA", bufs=psa_bufs, space="PSUM") as psA, \
             tc.tile_pool(name="psB", bufs=psb_bufs, space="PSUM") as psB:

            def load_group_x(g):
                """k-major interleaved DMA issue: the k0 slices of every
                tile in the group land first, so the fc (k, ti) matmul
                order is DMA-ready in emission order and the scheduler
                doesn't fall back to per-tile k-chains (which would defeat
                the shared-weight LDW elision)."""
                xs = [xpool.tile([P, KT, msz], DT16, tag="x", name="x_sb")
                      for _ in g]
                for k in range(KT):
                    for ti, (m0, mw) in enumerate(g):
                        nc.sync.dma_start(xs[ti][:, k, :mw],
                                          xT[k, :, m0:m0 + mw])
                return xs

            # First group's tokens queued ahead of the resident weights so
            # the PE can start as soon as the first wfc slab lands.
            pre_x = load_group_x(groups[0])

            bfc_sb = wpool.tile([P, FT], F32, tag="bfc", name="bfc_sb")
            nc.sync.dma_start(bfc_sb[:, :], bfcT[:, :])
            wproj_sb = wpool.tile([P, FT, N_EMBD], DT16, tag="wproj",
                                  name="wproj_sb")
            for f in range(FT):
                nc.sync.dma_start(wproj_sb[:, f, :], wproj[f, :, :])

            for _r in range(repeat):
                for gi, g in enumerate(groups):
                    if _r == 0 and gi == 0:
                        xs = pre_x
                    else:
                        xs = load_group_x(g)
                    hs = [hpool.tile([P, FT, msz], DT16, tag=f"h{ti}", name=f"h{ti}_sb")
                          for ti in range(len(g))]

                    for f in range(FT):
                        wf = wfpool.tile([P, KT * P], DT16, tag="wf", name="wf_sb")
                        nc.sync.dma_start(wf[:, :], wfcT[f, :, :])
                        pss = [psA.tile([P, msz], F32, tag="psA", name="psA_t")
                               for _ in g]
                        for k in range(KT):
                            for ti, (m0, mw) in enumerate(g):
                                nc.tensor.matmul(
                                    pss[ti][:, :mw],
                                    wf[:, k * P:(k + 1) * P],
                                    xs[ti][:, k, :mw],
                                    start=(k == 0),
                                    stop=(k == KT - 1),
                                )
                        for ti, (m0, mw) in enumerate(g):
                            nc.scalar.activation(
                                hs[ti][:, f, :mw], pss[ti][:, :mw],
                                mybir.ActivationFunctionType.Gelu,
                                bias=bfc_sb[:, f:f + 1],
                            )

                    for c in range(CT):
                        pss = [psB.tile([P, msz], F32, tag="psB", name="psB_t")
                               for _ in g]
                        for f in range(FT):
                            for ti, (m0, mw) in enumerate(g):
                                nc.tensor.matmul(
                                    pss[ti][:, :mw],
                                    wproj_sb[:, f, c * P:(c + 1) * P],
                                    hs[ti][:, f, :mw],
                                    start=(f == 0),
                                    stop=(f == FT - 1),
                                )
                        for ti, (m0, mw) in enumerate(g):
                            o_sb = opool.tile([P, msz], F32, tag="o", name="o_sb")
                            nc.vector.tensor_copy(o_sb[:, :mw], pss[ti][:, :mw])
                            nc.sync.dma_start(outT[c, :, m0:m0 + mw],
                                              o_sb[:, :mw])

    _optimize_pe_stream(nc, dedup=dedup, incbatch=incbatch)
    nc.compile()
    return nc


def _route(x_flat, router_w):
    """Top-2 routing, matching the reference numerics (f32)."""
    N = x_flat.shape[0]
    logits = x_flat @ router_w.T                      # [N, E]
    logits -= logits.max(axis=-1, keepdims=True)
    p = np.exp(logits)
    p /= p.sum(axis=-1, keepdims=True)
    rows = np.arange(N)
    i1 = p.argmax(axis=-1)
    p1 = p[rows, i1]
    pm = p.copy()
    pm[rows, i1] = -1.0
    i2 = pm.argmax(axis=-1)
    p2 = p[rows, i2]
    s = p1 + p2 + 1e-9
    return i1, i2, p1 / s, p2 / s


def _prep_in_maps(x_flat, idxs, w_fc, b_fc, w_proj, M):
    """Per-core input dict for core e = expert e (gathered, padded, f16)."""
    C = x_flat.shape[1]
    in_maps = []
    for e in range(N_EXPERTS):
        idx = idxs[e]
        xg = np.zeros((M, C), dtype=np.float32)
        xg[: len(idx)] = x_flat[idx]
        wfcT = np.ascontiguousarray(
            w_fc[e].reshape(KT, P, FT, P).transpose(2, 1, 0, 3)
        ).reshape(FT, P, KT * P).astype(np.float16)
        in_maps.append({
            "xT": np.ascontiguousarray(xg.T).reshape(KT, P, M).astype(np.float16),
            "wfcT": wfcT,
            "b_fcT": np.ascontiguousarray(b_fc[e].reshape(FT, P).T),
            "w_proj": w_proj[e].reshape(FT, P, N_EMBD).astype(np.float16),
        })
    return in_maps


def kernel(x, router_w, w_fc, b_fc, w_proj, b_proj):
    x = np.asarray(x, dtype=np.float32)
    router_w = np.asarray(router_w, dtype=np.float32)
    w_fc = np.asarray(w_fc, dtype=np.float32)
    b_fc = np.asarray(b_fc, dtype=np.float32)
    w_proj = np.asarray(w_proj, dtype=np.float32)
    b_proj = np.asarray(b_proj, dtype=np.float32)

    B, T, C = x.shape
    x_flat = x.reshape(-1, C)
    N = x_flat.shape[0]

    i1, i2, g1, g2 = _route(x_flat, router_w)

    idxs, gates = [], []
    for e in range(N_EXPERTS):
        mask = (i1 == e) | (i2 == e)
        idx = np.flatnonzero(mask)
        g = np.where(i1[idx] == e, g1[idx], g2[idx]).astype(np.float32)
        idxs.append(idx)
        gates.append(g)

    max_cnt = max(len(ix) for ix in idxs)
    M = max(P, ((max_cnt + P - 1) // P) * P)

    nc = _build(M)
    in_maps = _prep_in_maps(x_flat, idxs, w_fc, b_fc, w_proj, M)
    res = run_bass_kernel_spmd(nc, in_maps, core_ids=list(range(N_CORES)))

    out_flat = np.zeros((N, C), dtype=np.float32)
    for e in range(N_EXPERTS):
        idx = idxs[e]
        yT = res.results[e]["outT"].reshape(C, M)       # [C, M]
        y = yT.T[: len(idx)]                            # [n_e, C]
        out_flat[idx] += gates[e][:, None] * (y + b_proj[e])

    return out_flat.reshape(B, T, C)
